# revision 1
# baseline (speedup 1.0000x reference)
"""YOLOv1 loss kernel for Trainium2, data-parallel over 8 NeuronCores.

Full inputs: pred [16384,30,7,7] f32, labels [16384,30,7,7] f32 -> scalar f32.

Sharding: batch 16384 -> 8 cores x 2048 rows. Host packs each core's shard
into bf16 channel-major images (geo: 19 channels, cls: 40 channels) so every
on-chip op is one wide contiguous span (DVE 2x bf16 mode). Per core the
kernel streams NCHUNK chunks, computes the loss fully on chip into two f32
accumulators per chunk ([P] each), and the host sums partials / divides by B.

Math (equivalent to the reference up to rounding):
  - Grid offsets m,n cancel in the IOU; scaling coords by 7 cancels in
    inter/union. Interval overlap identity: min(hi)-max(lo) =
    3.5*(wp+wg) - |xp-xg|, so no lo/hi box corners are ever formed.
  - inter = relu(ovx)*relu(ovy), den = 49*(a+ag) - inter, iou = inter/den
    (den >= 49*ag > 0 always).  1/den via exp(-ln(den)) on ACT.
  - (sqrt(p)-sqrt(l))^2 = p + l - 2*sqrt(p*l): e5 = 5*(p+l) - 10*sqrt(p*l),
    with 10*sqrt(m) = exp(0.5*ln(100*m)).
  - Per cell: cell = sph + obj*(selU + S + cls - sph), sph = 0.5(c1^2+c2^2),
    selU = resp ? cc1+0.5objc1 : cc2+0.5objc2, S = 0.5objc1+0.5objc2.
    Sum(sph) via ACT accumulate; Sum(obj*(...)) via tensor_tensor_reduce.
"""

import numpy as np
from ml_dtypes import bfloat16

import concourse.bass as bass
import concourse.mybir as mybir
import concourse.tile as tile
from concourse import bacc
from concourse.bass_utils import run_bass_kernel_spmd

F32 = mybir.dt.float32
BF16 = mybir.dt.bfloat16
I32 = mybir.dt.int32
OP = mybir.AluOpType
AF = mybir.ActivationFunctionType

NCORES = 8
B = 16384
BLOC = B // NCORES        # 2048 rows per core
P = 128                   # SBUF partitions
KS = [4, 12]              # rows per partition per chunk (sum = BLOC/P)
NCHUNK = len(KS)
CS = [49 * k for k in KS]  # cells per partition per chunk
GCH = 19                  # geo channels
CCH = 40                  # cls channels (20 pred + 20 label)

SQ5 = float(np.float32(np.sqrt(5.0)))
ISQ2 = float(np.float32(np.sqrt(0.5)))

# how many cls channels the GPSIMD engine subtracts (rest on DVE)
GPS_SUB_CH = 4
# cls channels reduced by GPSIMD pool (window POOLW); rest tree-added on DVE
POOLW = 10


def _body(tc, geo_ap, cls_ap, out_ap):
    nc = tc.nc
    nv = nc.vector
    na = nc.scalar
    ng = nc.gpsimd

    import contextlib
    ctx = contextlib.ExitStack()
    with ctx:
        inp = ctx.enter_context(tc.tile_pool(name="inp", bufs=1))
        med = ctx.enter_context(tc.tile_pool(name="med", bufs=1))
        opool = ctx.enter_context(tc.tile_pool(name="opool", bufs=1))

        acc = opool.tile([P, 2 * NCHUNK], F32)
        CMAX = max(CS)

        GTs = [inp.tile([P, GCH * CS[i]], BF16, tag=f"GT{i}", name=f"GT{i}")
               for i in range(NCHUNK)]
        LTs = [inp.tile([P, CCH * CS[i]], BF16, tag=f"LT{i}", name=f"LT{i}")
               for i in range(NCHUNK)]

        def issue_dma(i, eng):
            Ci = CS[i]
            go = GCH * sum(CS[:i])
            lo = CCH * sum(CS[:i])
            eng.dma_start(GTs[i][:, 0:8 * Ci], geo_ap[:, go:go + 8 * Ci])
            eng.dma_start(GTs[i][:, 8 * Ci:GCH * Ci],
                          geo_ap[:, go + 8 * Ci:go + GCH * Ci])
            eng.dma_start(LTs[i][:, 0:CCH * Ci],
                          cls_ap[:, lo:lo + CCH * Ci])

        issue_dma(0, nc.sync)

        for c in range(NCHUNK):
            C = CS[c]
            GT = GTs[c]
            LT = LTs[c]

            def g(a, b):
                # geo channel span [a, b)
                return GT[:, a * C:b * C]

            def t2(name, ch, pool=med, dt=BF16):
                t = pool.tile([P, ch * CMAX], dt, tag=name, name=name)
                return t[:, 0:ch * C]

            # ---- geometry ----
            D4 = t2("D4", 4)     # [x1-lx, x2-lx, y1-ly, y2-ly]
            nv.tensor_tensor(D4[:], g(0, 4), g(4, 8), OP.subtract)
            A4 = t2("A4", 4)     # |dxy|/3.5 -> ov -> rv (in place)
            na.activation(A4[:], D4[:], AF.Abs, scale=1.0 / 3.5)
            # w/h channel pair views: [[w1 w2],[h1 h2]] and [[lw lw],[lh lh]]
            gv = GT[:, 8 * C:16 * C].rearrange("p (b x) -> p b x", b=2)
            wh_p = gv[:, :, 0:2 * C]
            wh_l = gv[:, :, 2 * C:4 * C]

            def bv(t):
                return t.rearrange("p (b x) -> p b x", b=2)

            T4 = t2("T4", 4)     # [w1+lw, w2+lw, h1+lh, h2+lh]
            nv.tensor_tensor(bv(T4), wh_p, wh_l, OP.add)
            M = t2("M", 4)       # [w1*lw, w2*lw, h1*lh, h2*lh]
            nv.tensor_tensor(bv(M), wh_p, wh_l, OP.mult)
            AAG = t2("AAG", 4)   # [a1, a2, ag, ag]
            nv.tensor_tensor(AAG[:], g(8, 12), g(12, 16), OP.mult)

            # 2*sqrt(m) = sqrt(4*m)   (in place over M)
            na.activation(M[:], M[:], AF.Sqrt, scale=4.0)
            if c + 1 < NCHUNK:
                # issue next chunk's loads from the ACT sequencer so they
                # start only now, giving chunk c's transfers full bandwidth
                issue_dma(c + 1, nc.scalar)

            # ---- intersection / iou ----
            # rv = min(relu(t - |dxy|/3.5)/2, min(wp,wg)) = overlap/7
            MN = t2("MN", 4)
            nv.tensor_tensor(bv(MN), wh_p, wh_l, OP.min)
            nv.tensor_tensor(A4[:], T4[:], A4[:], OP.subtract)
            nv.tensor_scalar(A4[:], A4[:], 0.0, 0.5, OP.max, OP.mult)
            nv.tensor_tensor(A4[:], A4[:], MN[:], OP.min)
            I2 = t2("I2", 2)     # inter/49 -> iou (in place)
            nv.tensor_tensor(I2[:], A4[:, 0:2 * C], A4[:, 2 * C:4 * C],
                             OP.mult)
            SSB = t2("SSB", 2)   # a + ag (bf16)
            nv.tensor_tensor(SSB[:], AAG[:, 0:2 * C], AAG[:, 2 * C:4 * C],
                             OP.add)
            S2 = t2("S2", 2, dt=F32)  # den/49 -> 1/den (in place, fp32)
            nv.tensor_tensor(S2[:], SSB[:], I2[:], OP.subtract)
            nv.reciprocal_approx_fast(S2[:], S2[:])
            RCB = t2("RCB", 2)   # 1/den cast to bf16 (on ACT)
            na.activation(RCB[:], S2[:], AF.Copy)
            nv.tensor_tensor(I2[:], I2[:], RCB[:], OP.mult)   # iou1, iou2

            RSP = t2("RSP", 1, dt=I32)
            nv.tensor_tensor(RSP[:], I2[:, 0:C], I2[:, C:2 * C], OP.is_ge)

            # objc'' = 0.1*(c - iou)^2
            DC = t2("DC", 2)
            nv.tensor_tensor(DC[:], g(16, 18), I2[:], OP.subtract)
            na.activation(DC[:], DC[:], AF.Square,
                          scale=float(np.sqrt(0.1)))

            # ---- coordinate loss (unscaled; the x5 is folded in at w2) ----
            DQ = t2("DQ", 8)     # [dxy^2 (4ch) | t - 2*sqrt(m) (4ch)]
            na.activation(DQ[:, 0:4 * C], D4[:], AF.Square)
            nv.tensor_tensor(DQ[:, 4 * C:8 * C], T4[:], M[:], OP.subtract)
            nv.tensor_tensor(DQ[:, 0:4 * C], DQ[:, 0:4 * C],
                             DQ[:, 4 * C:8 * C], OP.add)
            nv.tensor_tensor(DQ[:, 0:2 * C], DQ[:, 0:2 * C],
                             DQ[:, 2 * C:4 * C], OP.add)    # coor1, coor2
            nv.tensor_tensor(DQ[:, 0:2 * C], DQ[:, 0:2 * C], DC[:],
                             OP.add)                # coor_k + objc''_k

            SS1 = t2("SS1", 1)   # S'' = objc''1 + objc''2
            nv.tensor_tensor(SS1[:], DC[:, 0:C], DC[:, C:2 * C], OP.add)

            # sph = 0.5*(c1^2 + c2^2) per cell; noobj = 1 - obj
            H2 = t2("H2", 2)
            na.activation(H2[:], g(16, 18), AF.Square, scale=ISQ2)
            SPH = t2("SPH", 1)
            nv.tensor_tensor(SPH[:], H2[:, 0:C], H2[:, C:2 * C], OP.add)
            NOB = t2("NOB", 1)
            nv.tensor_scalar(NOB[:], g(18, 19), -1.0, 1.0, OP.mult, OP.add)

            # ---- cls: two interleaved 10-channel pipelines ----
            D20 = t2("D20", 20)
            oo = [0, 10 * C]

            def lvl(f):
                for o in oo:
                    f(o)

            lvl(lambda o: nv.tensor_tensor(
                D20[:, o:o + 10 * C], LT[:, o:o + 10 * C],
                LT[:, 20 * C + o:30 * C + o], OP.subtract))
            lvl(lambda o: na.activation(
                D20[:, o:o + 10 * C], D20[:, o:o + 10 * C], AF.Square))
            lvl(lambda o: nv.tensor_tensor(
                D20[:, o:o + 5 * C], D20[:, o:o + 5 * C],
                D20[:, o + 5 * C:o + 10 * C], OP.add))
            lvl(lambda o: nv.tensor_tensor(
                D20[:, o:o + 2 * C], D20[:, o:o + 2 * C],
                D20[:, o + 2 * C:o + 4 * C], OP.add))
            lvl(lambda o: nv.tensor_tensor(
                D20[:, o:o + C], D20[:, o:o + C],
                D20[:, o + C:o + 2 * C], OP.add))
            lvl(lambda o: nv.tensor_tensor(
                D20[:, o:o + C], D20[:, o:o + C],
                D20[:, o + 4 * C:o + 5 * C], OP.add))
            nv.tensor_tensor(D20[:, 0:C], D20[:, 0:C],
                             D20[:, 10 * C:11 * C], OP.add)  # cls total

            # ---- combine:
            # acc_a += sum(noobj * sph)
            # acc_b += sum(obj * (5*(sel(resp, cc1, cc2) + S'') + cls))
            V1 = t2("V1", 1)
            na.activation(V1[:], DQ[:, C:2 * C], AF.Copy)
            nv.copy_predicated(V1[:], RSP[:], DQ[:, 0:C])
            SC1 = t2("SC1", 1)
            nv.tensor_tensor(SC1[:], SPH[:], NOB[:], OP.mult)
            nv.tensor_reduce(acc[:, 2 * c:2 * c + 1], SC1[:],
                             mybir.AxisListType.X, OP.add)
            nv.tensor_tensor(V1[:], V1[:], SS1[:], OP.add)
            nv.scalar_tensor_tensor(V1[:], V1[:], 5.0, D20[:, 0:C],
                                    OP.mult, OP.add)
            W4 = t2("W4", 1)
            nv.tensor_tensor(W4[:], V1[:], g(18, 19), OP.mult)
            nv.tensor_reduce(acc[:, 2 * c + 1:2 * c + 2], W4[:],
                             mybir.AxisListType.X, OP.add)

        nc.sync.dma_start(out_ap, acc[:])


_NC_CACHE = None


def build_nc():
    global _NC_CACHE
    if _NC_CACHE is not None:
        return _NC_CACHE
    nc = bacc.Bacc(
        "TRN2",
        target_bir_lowering=False,
        debug=False,
        enable_asserts=False,
        num_devices=NCORES,
    )
    CT = sum(CS)
    geo = nc.dram_tensor("geo", [P, GCH * CT], BF16, kind="ExternalInput")
    cls = nc.dram_tensor("cls", [P, CCH * CT], BF16, kind="ExternalInput")
    out = nc.dram_tensor("out", [P, 2 * NCHUNK], F32, kind="ExternalOutput")
    with tile.TileContext(nc) as tc:
        _body(tc, geo.ap(), cls.ap(), out.ap())
    nc.compile()
    _NC_CACHE = nc
    return nc


def make_in_maps(pred, labels):
    pred = np.asarray(pred, dtype=np.float32).reshape(B, 30, 49)
    labels = np.asarray(labels, dtype=np.float32).reshape(B, 30, 49)
    pg = [0, 5, 1, 6,          # x1 x2 y1 y2
          2, 7,                # w1 w2
          3, 8,                # h1 h2
          4, 9]                # c1 c2
    geo_parts, cls_parts = [], []
    r0 = 0
    for c, k in enumerate(KS):
        Cc = CS[c]
        rows = P * k
        # rows r0 .. r0+rows: partition p holds rows r0 + j*P + p, j<k
        def img(x, chans):
            n = len(chans)
            y = x[:, chans].reshape(NCORES, BLOC, n, 49)
            y = y[:, r0:r0 + rows].reshape(NCORES, k, P, n, 49)
            y = y.transpose(0, 2, 3, 1, 4)    # core, p, ch, k, 49
            return np.ascontiguousarray(y).reshape(NCORES, P, n, Cc)

        gp = img(pred, pg)
        gl = img(labels, [0, 1, 2, 3, 4])
        geo = np.empty((NCORES, P, GCH, Cc), dtype=np.float32)
        geo[:, :, 0:4] = gp[:, :, 0:4]                   # x1 x2 y1 y2
        geo[:, :, 4] = gl[:, :, 0]                       # lx
        geo[:, :, 5] = gl[:, :, 0]                       # lx
        geo[:, :, 6] = gl[:, :, 1]                       # ly
        geo[:, :, 7] = gl[:, :, 1]                       # ly
        geo[:, :, 8:10] = gp[:, :, 4:6]                  # w1 w2
        geo[:, :, 10] = gl[:, :, 2]                      # lw
        geo[:, :, 11] = gl[:, :, 2]                      # lw
        geo[:, :, 12:14] = gp[:, :, 6:8]                 # h1 h2
        geo[:, :, 14] = gl[:, :, 3]                      # lh
        geo[:, :, 15] = gl[:, :, 3]                      # lh
        geo[:, :, 16:18] = gp[:, :, 8:10]                # c1 c2
        geo[:, :, 18] = gl[:, :, 4]                      # obj
        geo_parts.append(geo.reshape(NCORES, P, GCH * Cc))
        clsb = np.concatenate(
            [img(pred, list(range(10, 30))),
             img(labels, list(range(10, 30)))], axis=2)
        cls_parts.append(clsb.reshape(NCORES, P, CCH * Cc))
        r0 += rows
    geo = np.concatenate(geo_parts, axis=2).astype(bfloat16)
    cls = np.concatenate(cls_parts, axis=2).astype(bfloat16)
    return [
        {"geo": np.ascontiguousarray(geo[i]),
         "cls": np.ascontiguousarray(cls[i])}
        for i in range(NCORES)
    ]


def run(pred, labels, trace=False, **kw):
    nc = build_nc()
    in_maps = make_in_maps(pred, labels)
    res = run_bass_kernel_spmd(
        nc, in_maps, core_ids=list(range(NCORES)), trace=trace, **kw)
    total = np.float64(0.0)
    for r in res.results:
        total += r["out"].astype(np.float64).sum()
    loss = np.float32(total / B)
    return loss, res


def kernel(pred, labels):
    loss, _ = run(pred, labels)
    return np.array(loss, dtype=np.float32)



# revision 12
# speedup vs baseline: 1.1110x; 1.1110x over previous
"""YOLOv1 loss kernel for Trainium2, data-parallel over 8 NeuronCores.

Full inputs: pred [16384,30,7,7] f32, labels [16384,30,7,7] f32 -> scalar f32.

v4: chunk 0 ships as bf16 over HWDGE (zero Q7 descriptor-gen cost) so DVE
compute starts as soon as its ~2-3MB land; the big remaining chunk ships as
int8 (half the HBM bytes): one SWDGE cast DMA for the 19 geo channels and a
CCE-accumulate of the negated cls labels onto the int8 cls-pred tile (the
only CCE user; CCE descriptors are capped at 2048 elements and need 64B
aligned sources, so slices + per-chunk DRAM tensors).

Quantization (pure per-channel fixed-point casts, labels negated where only
differences are consumed): q=127 for w/h/c/cls/obj, q=127/3.5 for x/y so
the IOU |dx|/3.5 comparison needs no extra scaling.

SBUF geo channel map GB[19]: [x1 x2 y1 y2 w1 w2 h1 h2 | -lx -lx -ly -ly
lw lw lh lh | c1 c2 obj] so D4|T4 = GB[0:8]+GB[8:16] is one fused DVE add,
and M4/MN pair channel-wise without rearranged views.

Math (= reference up to quantization):
  overlap = 0.5*[(w+lw) - |dx|/3.5] capped by min(w,lw); iou = I/(a+ag-I);
  (sqrt(w)-sqrt(lw))^2 = (w+lw) - 2*sqrt(w*lw);
  responsible-box select via is_ge mask arithmetic V = B2 + resp*(B1-B2);
  noobj term: sum_all 0.5(c1^2+c2^2) via ACT Square accum_out, the per-cell
  sph is subtracted inside the obj-masked stt accumulation.
"""

import numpy as np
from ml_dtypes import bfloat16

import concourse.bass as bass
import concourse.mybir as mybir
import concourse.tile as tile
from concourse import bacc
from concourse.bass_utils import run_bass_kernel_spmd

F32 = mybir.dt.float32
BF16 = mybir.dt.bfloat16
I8 = mybir.dt.int8
OP = mybir.AluOpType
AF = mybir.ActivationFunctionType

NCORES = 8
B = 16384
BLOC = B // NCORES        # 2048 rows per core
P = 128                   # SBUF partitions
KS = [3, 13]              # rows per partition per chunk (sum = BLOC/P)
NCHUNK = len(KS)
CS = [49 * k for k in KS]  # cells per partition per chunk
BF_CHUNKS = {0}           # chunks shipped as bf16 over HWDGE

QW = 127.0
QX = QW / 3.5

# ---- tuning knobs ----
M4_GPS = False            # M4 product on GPSIMD
A2AG_GPS = False          # a1a2/agag products on GPSIMD
T2_GPS = False            # cls half-tree T2 on GPSIMD

SCL_SQ20 = float(1.0 / np.sqrt(5.0 * QW))
SCL_DQ03 = float(3.5 / np.sqrt(QW))
SCL_DC2 = float(np.sqrt(0.1 / QW))
SCL_ACC = float(5.0 / (QW * QW))


def _body(tc, aps, out_ap):
    nc = tc.nc
    nv = nc.vector
    na = nc.scalar
    ng = nc.gpsimd

    import contextlib
    ctx = contextlib.ExitStack()
    with ctx:
        inp = ctx.enter_context(tc.tile_pool(name="inp", bufs=1))
        med = ctx.enter_context(tc.tile_pool(name="med", bufs=1))
        opool = ctx.enter_context(tc.tile_pool(name="opool", bufs=1))

        acc = opool.tile([P, 2 * NCHUNK], F32)
        CMAX = max(CS)

        GBs = [inp.tile([P, 19 * CS[i]], BF16, name=f"GB{i}")
               for i in range(NCHUNK)]
        CLs = [inp.tile([P, 40 * CS[i]], BF16, name=f"CL{i}")
               if i in BF_CHUNKS else
               inp.tile([P, 20 * CS[i]], I8, name=f"CL{i}")
               for i in range(NCHUNK)]

        # ---- DMA issue ----
        # HWDGE (sync queue): chunk0 bf16 first (gates compute start), then
        # the int8 cls-pred tiles the CCE accumulates depend on.
        for i in range(NCHUNK):
            if i in BF_CHUNKS:
                nc.sync.dma_start(GBs[i][:], aps[i]["gb"])
                nc.sync.dma_start(CLs[i][:], aps[i]["cls"])
        for i in range(NCHUNK):
            if i not in BF_CHUNKS:
                nc.sync.dma_start(CLs[i][:], aps[i]["clp"])
        # SWDGE (gpsimd queue): cast of geo, CCE-add of negated cls labels.
        for i in range(NCHUNK):
            if i not in BF_CHUNKS:
                ng.dma_start(GBs[i][:], aps[i]["geob"])
        for i in range(NCHUNK):
            if i not in BF_CHUNKS:
                n = 20 * CS[i]
                for s in range(0, n, 2048):
                    e = min(s + 2048, n)
                    ng.dma_start(CLs[i][:, s:e], aps[i]["clln"][:, s:e],
                                 accum_op=OP.add)

        for c in range(NCHUNK):
            C = CS[c]
            GB = GBs[c]
            CL = CLs[c]

            def g(a, b):
                return GB[:, a * C:b * C]

            def t2(name, ch, dt=BF16, pool=med):
                t = pool.tile([P, ch * CMAX], dt, tag=name, name=name)
                return t[:, 0:ch * C]

            WP4 = g(4, 8)       # w1 w2 h1 h2
            LP4 = g(12, 16)     # lw lw lh lh
            COBc = g(16, 18)    # c1 c2
            OBJ = g(18, 19)

            # ---- fused diffs/sums: [D4 | T4] = pred8 + label8n ----
            D8 = t2("D8", 8)
            nv.tensor_tensor(D8[:], g(0, 8), g(8, 16), OP.add)
            D4 = D8[:, 0:4 * C]
            T4 = D8[:, 4 * C:8 * C]

            A4 = t2("A4", 4)
            na.activation(A4[:], D4, AF.Abs)
            MN4 = t2("MN4", 4)
            nv.tensor_tensor(MN4[:], WP4, LP4, OP.min)
            M4 = t2("M4", 4)
            (ng if M4_GPS else nv).tensor_tensor(M4[:], WP4, LP4, OP.mult)

            # Z = T4 - A4 ; R = relu(Z)*0.5 ; OV = min(R, MN)
            nv.tensor_tensor(A4[:], T4, A4[:], OP.subtract)
            nv.tensor_scalar(A4[:], A4[:], 0.0, 0.5, OP.max, OP.mult)
            nv.tensor_tensor(A4[:], A4[:], MN4[:], OP.min)
            I2 = t2("I2", 2)
            nv.tensor_tensor(I2[:], A4[:, 0:2 * C], A4[:, 2 * C:4 * C],
                             OP.mult)                    # inter*127^2

            AAG = t2("AAG", 4)   # [a1 a2 | ag ag]
            ee = ng if A2AG_GPS else nv
            ee.tensor_tensor(AAG[:, 0:2 * C], g(4, 6), g(6, 8), OP.mult)
            ee.tensor_tensor(AAG[:, 2 * C:4 * C], g(12, 14), g(14, 16),
                             OP.mult)
            SSB = t2("SSB", 2)
            nv.tensor_tensor(SSB[:], AAG[:, 0:2 * C], AAG[:, 2 * C:4 * C],
                             OP.add)
            DEN = t2("DEN", 2, dt=F32)
            nv.tensor_tensor(DEN[:], SSB[:], I2[:], OP.subtract)
            nv.reciprocal_approx_fast(DEN[:], DEN[:])
            RCB = t2("RCB", 2)
            na.activation(RCB[:], DEN[:], AF.Copy, scale=float(QW))
            IOUS = t2("IOUS", 2)
            nv.tensor_tensor(IOUS[:], I2[:], RCB[:], OP.mult)   # 127*iou

            RSP = t2("RSP", 1)
            nv.tensor_tensor(RSP[:], IOUS[:, 0:C], IOUS[:, C:2 * C],
                             OP.is_ge)
            DC = t2("DC", 2)
            nv.tensor_tensor(DC[:], COBc, IOUS[:], OP.subtract)
            DC2 = t2("DC2", 2)
            na.activation(DC2[:], DC[:], AF.Square, scale=SCL_DC2)

            # ---- coordinate loss ----
            SQ4 = t2("SQ4", 4)
            na.activation(SQ4[:], M4[:], AF.Sqrt, scale=4.0)
            DQ = t2("DQ", 4)
            na.activation(DQ[:], D4, AF.Square, scale=SCL_DQ03)
            nv.tensor_tensor(SQ4[:], T4, SQ4[:], OP.subtract)
            nv.tensor_tensor(DQ[:], DQ[:], SQ4[:], OP.add)
            B2 = t2("B2", 2)
            nv.tensor_tensor(B2[:], DQ[:, 0:2 * C], DQ[:, 2 * C:4 * C],
                             OP.add)
            nv.tensor_tensor(B2[:], B2[:], DC2[:], OP.add)

            # ---- sph: ACT square with accumulate (positive) ----
            H2 = t2("H2", 2)
            na.activation(H2[:], COBc, AF.Square, scale=SCL_DC2,
                          accum_out=acc[:, 2 * c:2 * c + 1])
            X1 = t2("X1", 1)
            nv.tensor_tensor(X1[:], DC2[:, 0:C], DC2[:, C:2 * C], OP.add)
            HP = t2("HP", 1)
            nv.tensor_tensor(HP[:], H2[:, 0:C], H2[:, C:2 * C], OP.add)
            nv.tensor_tensor(X1[:], X1[:], HP[:], OP.subtract)

            # ---- cls ----
            if c in BF_CHUNKS:
                D20 = t2("D20", 20)
                nv.tensor_tensor(D20[:], CL[:, 0:20 * C],
                                 CL[:, 20 * C:40 * C], OP.add)
                cl_src = D20[:]
            else:
                cl_src = CL[:]
            SQ20 = t2("SQ20", 20)
            na.activation(SQ20[:, 0:10 * C], cl_src[:, 0:10 * C],
                          AF.Square, scale=SCL_SQ20)
            na.activation(SQ20[:, 10 * C:20 * C], cl_src[:, 10 * C:20 * C],
                          AF.Square, scale=SCL_SQ20)
            TR = t2("TR", 10)
            nv.tensor_tensor(TR[:, 0:5 * C], SQ20[:, 0:5 * C],
                             SQ20[:, 5 * C:10 * C], OP.add)
            (ng if T2_GPS else nv).tensor_tensor(
                TR[:, 5 * C:10 * C], SQ20[:, 10 * C:15 * C],
                SQ20[:, 15 * C:20 * C], OP.add)
            nv.tensor_tensor(TR[:, 0:5 * C], TR[:, 0:5 * C],
                             TR[:, 5 * C:10 * C], OP.add)
            nv.tensor_tensor(TR[:, 0:2 * C], TR[:, 0:2 * C],
                             TR[:, 2 * C:4 * C], OP.add)
            nv.tensor_tensor(TR[:, 0:C], TR[:, 0:C], TR[:, C:2 * C], OP.add)
            nv.tensor_tensor(TR[:, 0:C], TR[:, 0:C], TR[:, 4 * C:5 * C],
                             OP.add)                      # CLS'

            # ---- select + combine ----
            DB = t2("DB", 1)
            nv.tensor_tensor(DB[:], B2[:, 0:C], B2[:, C:2 * C], OP.subtract)
            nv.tensor_tensor(DB[:], DB[:], RSP[:], OP.mult)
            V1 = t2("V1", 1)
            nv.tensor_tensor(V1[:], B2[:, C:2 * C], DB[:], OP.add)
            nv.tensor_tensor(X1[:], X1[:], TR[:, 0:C], OP.add)
            nv.tensor_tensor(V1[:], V1[:], X1[:], OP.add)
            W4 = t2("W4", 1)
            nv.scalar_tensor_tensor(W4[:], V1[:], SCL_ACC, OBJ,
                                    OP.mult, OP.mult,
                                    accum_out=acc[:, 2 * c + 1:2 * c + 2])

        nc.sync.dma_start(out_ap, acc[:])


_NC_CACHE = None


def build_nc():
    global _NC_CACHE
    if _NC_CACHE is not None:
        return _NC_CACHE
    nc = bacc.Bacc(
        "TRN2",
        target_bir_lowering=False,
        debug=False,
        enable_asserts=False,
        num_devices=NCORES,
    )
    aps = []
    for i in range(NCHUNK):
        Ci = CS[i]
        if i in BF_CHUNKS:
            aps.append({
                "gb": nc.dram_tensor(f"gb{i}", [P, 19 * Ci], BF16,
                                     kind="ExternalInput").ap(),
                "cls": nc.dram_tensor(f"cls{i}", [P, 40 * Ci], BF16,
                                      kind="ExternalInput").ap(),
            })
        else:
            aps.append({
                "geob": nc.dram_tensor(f"geob{i}", [P, 19 * Ci], I8,
                                       kind="ExternalInput").ap(),
                "clp": nc.dram_tensor(f"clp{i}", [P, 20 * Ci], I8,
                                      kind="ExternalInput").ap(),
                "clln": nc.dram_tensor(f"clln{i}", [P, 20 * Ci], I8,
                                       kind="ExternalInput").ap(),
            })
    out = nc.dram_tensor("out", [P, 2 * NCHUNK], F32, kind="ExternalOutput")
    with tile.TileContext(nc) as tc:
        _body(tc, aps, out.ap())
    nc.compile()
    _NC_CACHE = nc
    return nc


def _q(x, scale):
    return np.clip(np.rint(x * scale), -127, 127).astype(np.int8)


def make_in_maps(pred, labels):
    pred = np.asarray(pred, dtype=np.float32).reshape(B, 30, 49)
    labels = np.asarray(labels, dtype=np.float32).reshape(B, 30, 49)

    def img(x, chans, k, r0):
        n = len(chans)
        y = x[:, chans].reshape(NCORES, BLOC, n, 49)
        y = y[:, r0:r0 + P * k].reshape(NCORES, k, P, n, 49)
        y = y.transpose(0, 2, 3, 1, 4)
        return np.ascontiguousarray(y).reshape(NCORES, P, n, 49 * k)

    streams = {}
    r0 = 0
    for c, k in enumerate(KS):
        Cc = CS[c]
        pg = img(pred, [0, 5, 1, 6, 2, 7, 3, 8, 4, 9], k, r0)
        lg = img(labels, [0, 1, 2, 3, 4], k, r0)
        pc = img(pred, list(range(10, 30)), k, r0)
        lc = img(labels, list(range(10, 30)), k, r0)

        geo = np.empty((NCORES, P, 19, Cc), dtype=np.float32)
        geo[:, :, 0:4] = pg[:, :, 0:4] * QX            # x1 x2 y1 y2
        geo[:, :, 4:8] = pg[:, :, 4:8] * QW            # w1 w2 h1 h2
        geo[:, :, 8] = -lg[:, :, 0] * QX               # -lx
        geo[:, :, 9] = geo[:, :, 8]
        geo[:, :, 10] = -lg[:, :, 1] * QX              # -ly
        geo[:, :, 11] = geo[:, :, 10]
        geo[:, :, 12] = lg[:, :, 2] * QW               # lw
        geo[:, :, 13] = geo[:, :, 12]
        geo[:, :, 14] = lg[:, :, 3] * QW               # lh
        geo[:, :, 15] = geo[:, :, 14]
        geo[:, :, 16:18] = pg[:, :, 8:10] * QW         # c1 c2
        geo[:, :, 18] = lg[:, :, 4] * QW               # obj

        if c in BF_CHUNKS:
            streams[f"gb{c}"] = geo.reshape(NCORES, P, 19 * Cc).astype(
                bfloat16)
            cl = np.concatenate([pc * QW, -lc * QW], axis=2)
            streams[f"cls{c}"] = cl.reshape(NCORES, P, 40 * Cc).astype(
                bfloat16)
        else:
            streams[f"geob{c}"] = np.clip(
                np.rint(geo), -127, 127).astype(np.int8).reshape(
                    NCORES, P, 19 * Cc)
            streams[f"clp{c}"] = _q(pc, QW).reshape(NCORES, P, 20 * Cc)
            streams[f"clln{c}"] = _q(-lc, QW).reshape(NCORES, P, 20 * Cc)
        r0 += P * k

    return [
        {k: np.ascontiguousarray(v[i]) for k, v in streams.items()}
        for i in range(NCORES)
    ]


def _reduce_core(a):
    # acc[:,0::2] = sum(0.1*c^2/QW) = 0.2*QW*sum(sph_real)
    # acc[:,1::2] = obj-masked per-cell sums (real units)
    a = np.asarray(a, dtype=np.float64)
    acc_s = a[:, 0::2].sum()
    acc_b = a[:, 1::2].sum()
    return acc_b + acc_s / (0.2 * QW)


def run(pred, labels, trace=False, **kw):
    nc = build_nc()
    in_maps = make_in_maps(pred, labels)
    res = run_bass_kernel_spmd(
        nc, in_maps, core_ids=list(range(NCORES)), trace=trace, **kw)
    total = np.float64(0.0)
    for r in res.results:
        total += _reduce_core(r["out"])
    loss = np.float32(total / B)
    return loss, res


def kernel(pred, labels):
    loss, _ = run(pred, labels)
    return np.array(loss, dtype=np.float32)
